# revision 22
# baseline (speedup 1.0000x reference)
"""3-layer GCN (DGL GraphConv, norm='both') on 8 Trainium2 NeuronCores.

Strategy (v3):
  - Nodes are packed into 80 balanced bins (128 slots each) by in-degree
    (greedy least-loaded), 10 bins per core -> 1280 padded rows/core.
  - Edges live with the owner (bin) of their dst node. segment_sum runs as
    "scatter matmuls" on the TensorEngine: for each dst block,
    agg[128d, D] += S_kt[128s, 128d].T @ msg_kt[128s, D]. msg rows are the
    DISTINCT src nodes of the block's edges (deduped, ordered by gathered-id
    so each chunk touches a narrow AllGather slab range); S carries
    sum-of-edge-weights norm_src[src]*norm_dst[dst] at (src_slot, dst_slot).
  - Layer 1 does NOT gather: the host pre-packs each core's msg rows
    (h[dedup src]) partition-major, streamed with large contiguous HWDGE
    descriptors. Only layer 2 uses SWDGE dma_gather (whose Q7 descriptor
    generation, ~8.5ns/row, is the scarce resource).
  - Layer-2 gather chunks restrict their source AP to the slab range their
    (gid-sorted) indices touch, with slab-relative indices, so early chunks
    fire while later AllGather stages are still in flight.
  - The on-device datapath is fp16 (PSUM accumulation fp32).
  - Dense W matmuls per dst block: PE-transpose agg -> aggT, then
    x = aggT.T @ W (+ bias via K=1 matmul), ReLU fused into PSUM->SBUF.
  - Layer 3 is PUSH-style, woven into layer 2's block loop (no gather, no
    3rd collective buffer): after y3_j = x2_j @ W3, stream this core's
    block-dense push matrix S'_j from HBM and accumulate
    partialT[64f, all 10240 dst] += y3_j.T-stationary @ S'_j into an SBUF
    fp16 accumulator (DVE adds). After the last block: PE-transpose the 80
    dst column-blocks, one ReduceScatter sums partials across cores, and
    each core writes its own 1280 output rows.
"""
import sys
sys.path.insert(0, '/opt/trn_rl_repo')
import numpy as np

N_CORES = 8
CHMAX = 5        # k-tiles per layer-2 gather chunk


def _ag_splits(nblk):
    """Block-index boundaries of the staged AllGather slabs."""
    if nblk <= 2:
        return [0, nblk]
    fr = [0, round(0.2 * nblk), round(0.4 * nblk), round(0.6 * nblk),
          round(0.8 * nblk), nblk - 1, nblk]
    return sorted(set(b for b in fr if 0 <= b <= nblk))


def _chunks(kt, chmax):
    """Split kt k-tiles into near-equal chunks of size <= chmax."""
    n = -(-kt // chmax)
    base, rem = divmod(kt, n)
    sizes = [base + 1] * rem + [base] * (n - rem)
    out, o = [], 0
    for s in sizes:
        out.append((o, s))
        o += s
    return out


# ---------------------------------------------------------------- host prep
def _partition_nodes(deg_in, n_nodes, nbins):
    """Greedy balanced-edge binning: nodes (sorted by in-degree desc) go to
    the least-loaded bin with a free slot (capacity 128)."""
    import heapq
    order = np.argsort(-deg_in, kind="stable")
    heap = [(0, b) for b in range(nbins)]
    heapq.heapify(heap)
    bin_of = np.empty(n_nodes, np.int32)
    slot_of = np.empty(n_nodes, np.int32)
    count = np.zeros(nbins, np.int64)
    load = np.zeros(nbins, np.int64)
    for n in order:
        while True:
            l, b = heapq.heappop(heap)
            if count[b] < 128:
                break
            # full bin: drop from heap permanently
        bin_of[n] = b
        slot_of[n] = count[b]
        count[b] += 1
        load[b] += int(deg_in[n])
        heapq.heappush(heap, (l + int(deg_in[n]), b))
    return bin_of, slot_of, load


def _prep(h, src, dst, cfg):
    """Build per-core S tiles, gather indices, slot->node maps, per-chunk
    slab ranges, and the layer-3 push matrices."""
    N, E, NBLK = cfg["N"], cfg["E"], cfg["NBLK"]
    nbins = N_CORES * NBLK
    deg_out = np.bincount(src, minlength=N)
    deg_in = np.bincount(dst, minlength=N)
    norm_src = np.clip(deg_out, 1, None).astype(np.float32) ** np.float32(-0.5)
    norm_dst = np.clip(deg_in, 1, None).astype(np.float32) ** np.float32(-0.5)
    w = (norm_src[src] * norm_dst[dst]).astype(np.float32)

    bin_of, slot_of, load = _partition_nodes(deg_in, N, nbins)

    # deal bins to cores snake-wise by load to balance core totals
    order = np.argsort(-load, kind="stable")
    core_of_bin = np.empty(nbins, np.int32)
    blk_of_bin = np.empty(nbins, np.int32)
    nextblk = [0] * N_CORES
    for i, b in enumerate(order):
        r = i // N_CORES
        c = (i % N_CORES) if r % 2 == 0 else (N_CORES - 1 - (i % N_CORES))
        core_of_bin[b] = c
        blk_of_bin[b] = nextblk[c]
        nextblk[c] += 1

    RPC = NBLK * 128
    row_of_node = (core_of_bin[bin_of] * RPC + blk_of_bin[bin_of] * 128
                   + slot_of).astype(np.int32)
    # gather-id layout after the staged slab AllGathers: slab q holds rows
    # [b_q, e_q) of every core, concatenated core-major at offset 8*b_q
    SPL = _ag_splits(NBLK)
    sp = np.array(SPL) * 128
    _c = row_of_node // RPC
    _r = row_of_node % RPC
    _q = np.searchsorted(sp, _r, side="right") - 1
    gid_of_node = (N_CORES * sp[_q] + _c * (sp[_q + 1] - sp[_q])
                   + _r - sp[_q]).astype(np.int32)

    # group edges by dst bin; dedup src nodes per bin, ordered by gid
    ebin = bin_of[dst]
    eorder = np.argsort(ebin, kind="stable")
    counts = np.bincount(ebin, minlength=nbins)
    bounds = np.concatenate([[0], np.cumsum(counts)])
    uniqs = []
    for b in range(nbins):
        es = eorder[bounds[b]:bounds[b + 1]]
        u, inv = np.unique(src[es], return_inverse=True)
        gorder = np.argsort(gid_of_node[u], kind="stable")
        rank = np.empty(len(u), np.int64)
        rank[gorder] = np.arange(len(u))
        uniqs.append((es, u[gorder], rank[inv]))
    max_u = max((len(u) for _, u, _ in uniqs), default=1)
    kt_blk = max(cfg["KT_MIN"], -(-max_u // 128))
    kt_tot = NBLK * kt_blk
    chunks = _chunks(kt_blk, CHMAX)

    # per-(block, chunk) slab-aligned source ranges (shared across cores)
    gbound = N_CORES * sp                     # gid-space slab boundaries
    glo = np.full((NBLK, len(chunks)), gbound[-1], np.int64)
    ghi = np.zeros((NBLK, len(chunks)), np.int64)
    gids_of = {}
    for b in range(nbins):
        es, u, inv = uniqs[b]
        c, blk = int(core_of_bin[b]), int(blk_of_bin[b])
        g = gid_of_node[u]
        gids_of[b] = g
        for ci, (c0, ch) in enumerate(chunks):
            seg = g[c0 * 128:(c0 + ch) * 128]
            if len(seg):
                glo[blk, ci] = min(glo[blk, ci], seg[0])
                ghi[blk, ci] = max(ghi[blk, ci], seg[-1] + 1)
    ranges = []
    for blk in range(NBLK):
        rr = []
        for ci in range(len(chunks)):
            lo = int(gbound[np.searchsorted(gbound, glo[blk, ci],
                                            side="right") - 1])
            hi = int(gbound[np.searchsorted(gbound, ghi[blk, ci],
                                            side="left")])
            if hi <= lo:
                hi = lo + 1024
            rr.append((lo, hi))
        ranges.append(rr)

    nodes_slot = np.zeros((N_CORES, kt_tot * 128), np.int32)
    idx23 = np.zeros((N_CORES, kt_tot * 128), np.int16)
    for blk in range(NBLK):
        for ci, (c0, ch) in enumerate(chunks):
            p0 = blk * kt_blk * 128 + c0 * 128
            idx23[:, p0:p0 + ch * 128] = 0  # relative pad = range base
    S = np.zeros((N_CORES, 128, kt_tot, 128), np.float32)
    for b in range(nbins):
        es, u, inv = uniqs[b]
        c, blk = int(core_of_bin[b]), int(blk_of_bin[b])
        g = gids_of[b]
        p = np.arange(len(u))
        gpos = blk * kt_blk * 128 + p
        nodes_slot[c, gpos] = u
        # slab-relative gather indices per chunk
        rel = np.empty(len(u), np.int64)
        for ci, (c0, ch) in enumerate(chunks):
            s0, s1 = c0 * 128, min((c0 + ch) * 128, len(u))
            if s1 > s0:
                rel[s0:s1] = g[s0:s1] - ranges[blk][ci][0]
        idx23[c, gpos] = rel.astype(np.int16)
        kt = blk * kt_blk + inv // 128
        np.add.at(S[c], (inv % 128, kt, slot_of[dst[es]]), w[es])
    S = S.astype(np.float16)

    # layer-3 push matrices: S'[c][j, src_slot, ob, dst_slot] sums w over
    # edges with src in (core c, block j) and dst in output block ob
    srow = row_of_node[src]
    drow = row_of_node[dst]
    c_u = srow // RPC
    j_u = (srow // 128) % NBLK
    s_u = srow % 128
    ob_v = drow // 128
    d_v = drow % 128
    sp3 = []
    for c in range(N_CORES):
        m = c_u == c
        spc = np.zeros((NBLK, 128, nbins, 128), np.float32)
        np.add.at(spc, (j_u[m], s_u[m], ob_v[m], d_v[m]), w[m])
        sp3.append(spc.astype(np.float16))

    def wrap(ix):  # -> [128, kt_tot*8] wrapped for the 8 Q7 cores
        return np.tile(ix.reshape(-1, 16).T, (8, 1)).copy()

    idx23_w = np.stack([wrap(idx23[c]) for c in range(N_CORES)])
    return dict(S=S, idx23=idx23_w, nodes_slot=nodes_slot, sp3=sp3,
                row_of_node=row_of_node, kt_blk=kt_blk, kt_tot=kt_tot,
                ranges=tuple(tuple(r) for r in ranges))


# ---------------------------------------------------------------- device prog
def _build(cfg, kt_blk, use_bias, ranges):
    import concourse.bacc as bacc
    import concourse.mybir as mybir
    import concourse.tile as tile
    from concourse.library_config import mlp

    f32 = mybir.dt.float32
    f16 = mybir.dt.float16
    i16 = mybir.dt.int16
    RELU = mybir.ActivationFunctionType.Relu
    COPY = mybir.ActivationFunctionType.Copy

    N, D, C, NBLK = cfg["N"], cfg["D"], cfg["C"], cfg["NBLK"]
    NBINS = N_CORES * NBLK
    RPC = NBLK * 128
    NPAD = N_CORES * RPC
    KT = kt_blk
    KT_TOT = NBLK * KT
    CHUNKS = _chunks(KT, CHMAX)
    KD = D // 128               # dense contraction k-tiles
    ND = 512 if D % 512 == 0 else D
    NT = D // ND                # dense n-tiles
    TPW = min(512, D)           # transposes packed per tps tile
    TPG = TPW // 128
    SPL = _ag_splits(NBLK)
    SPW = 512 // 128            # layer-3 push: output blocks per matmul
    NSP = NBINS // SPW          # layer-3 matmuls per source block
    SPH = 2                     # S'_j streamed in halves

    nc = bacc.Bacc("TRN2", target_bir_lowering=False, debug=False,
                   num_devices=N_CORES, num_swdge_queues=4,
                   dynamic_dma_scratch_size=32768)

    hxp = nc.dram_tensor("hxp", [128, KT_TOT, D], f16, kind="ExternalInput")
    sker = nc.dram_tensor("sker", [128, KT_TOT, 128], f16, kind="ExternalInput")
    idx23_h = nc.dram_tensor("idx23", [128, KT_TOT * 8], i16, kind="ExternalInput")
    w12_h = nc.dram_tensor("w12", [2, 128, KD, D], f16, kind="ExternalInput")
    w3_h = nc.dram_tensor("w3", [128, KD, C], f16, kind="ExternalInput")
    ident_h = nc.dram_tensor("ident", [128, 128], f16, kind="ExternalInput")
    bias_h = nc.dram_tensor("biases", [1, 2 * D + C + 128], f16, kind="ExternalInput")
    b3c_h = nc.dram_tensor("b3c", [C, 1], f32, kind="ExternalInput")
    sp3_h = nc.dram_tensor("sp3", [NBLK, 128, NBINS, 128], f16,
                           kind="ExternalInput")
    out_h = nc.dram_tensor("out", [RPC, C], f32, kind="ExternalOutput")

    ag_in = nc.dram_tensor("ag_in", [RPC, D], f16, kind="Internal")
    ag_out = nc.dram_tensor("ag_out", [NPAD, D], f16, kind="Internal",
                            addr_space="Shared")
    part_h = nc.dram_tensor("part", [NPAD, C], f16, kind="Internal")
    rs_out = nc.dram_tensor("rs_out", [RPC, C], f16, kind="Internal")

    with tile.TileContext(nc) as tc:
        nc.gpsimd.load_library(mlp)
        with (
            tc.tile_pool(name="const", bufs=1) as cp,
            tc.tile_pool(name="work", bufs=2) as wp,
            tc.tile_pool(name="msg", bufs=3) as mp,
            tc.tile_pool(name="spstream", bufs=2) as spp,
            tc.tile_pool(name="aggps", bufs=2, space="PSUM") as aps,
            tc.tile_pool(name="densps", bufs=2, space="PSUM") as dps,
            tc.tile_pool(name="tpsps", bufs=2, space="PSUM") as tps,
        ):
            ident_t = cp.tile([128, 128], f16, tag="ident")
            nc.sync.dma_start(ident_t[:], ident_h[:])
            s_blk = [cp.tile([128, KT, 128], f16, tag=f"s{b}", name=f"s_{b}")
                     for b in range(NBLK)]
            nc.sync.dma_start(s_blk[0][:], sker[:, 0:KT, :])
            w_t = cp.tile([128, KD, D], f16, tag="w")
            idx23_t = cp.tile([128, KT_TOT * 8], i16, tag="idx23")
            w3_t = cp.tile([128, KD, C], f16, tag="w3")
            acc_t = cp.tile([64, NBINS * 128], f16, tag="acc")
            aggblk = [cp.tile([128, D], f16, tag=f"ab{b}", name=f"ab_{b}")
                      for b in range(NBLK)]
            if use_bias:
                brow_t = cp.tile([1, 2 * D + C + 128], f16, tag="brow")
                nc.sync.dma_start(brow_t[:], bias_h[:])
                ones_t = brow_t[:, 2 * D + C:2 * D + C + 128]
                b3c_t = cp.tile([C, 1], f32, tag="b3c")
                nc.sync.dma_start(b3c_t[:], b3c_h[:])

            qctr = [0]

            def spmm_block(b):
                """L1: agg[128, D] for dst block b via packed streams + MMs."""
                agg = aps.tile([128, D], f32, tag="aggps")
                nspl = max(1, D // 512)
                for ci, (c0, ch) in enumerate(CHUNKS):
                    msg = mp.tile([128, CHMAX, D], f16, tag="m")
                    nc.sync.dma_start(
                        msg[:, :ch, :],
                        hxp[:, b * KT + c0:b * KT + c0 + ch, :])
                    for k in range(ch):
                        first = (c0 == 0 and k == 0)
                        last = (c0 + ch == KT and k == ch - 1)
                        for n in range(nspl):
                            w0 = n * (D // nspl)
                            w1 = (n + 1) * (D // nspl)
                            nc.tensor.matmul(
                                agg[:, w0:w1], s_blk[b][:, c0 + k, :],
                                msg[:, k, w0:w1],
                                start=first, stop=last)
                return agg

            def l2_chunk(b, ci):
                """L2 chunk: gather (slab-restricted) + MMs, accumulate into
                the block's SBUF partial."""
                c0, ch = CHUNKS[ci]
                agp = aps.tile([128, D], f32, tag="aggps")
                msg = mp.tile([128, CHMAX, D], f16, tag="m")
                lo, hi = ranges[b][ci]
                col0 = (b * KT + c0) * 8
                q = qctr[0] % 4
                qctr[0] += 1
                nc.gpsimd.dma_gather(
                    msg[:, :ch, :], ag_out[lo:hi, :],
                    idx23_t[:, col0:col0 + ch * 8],
                    ch * 128, ch * 128, D, queue_num=q)
                nspl = max(1, D // 512)
                for k in range(ch):
                    for n in range(nspl):
                        w0 = n * (D // nspl)
                        w1 = (n + 1) * (D // nspl)
                        nc.tensor.matmul(
                            agp[:, w0:w1], s_blk[b][:, c0 + k, :],
                            msg[:, k, w0:w1],
                            start=(k == 0), stop=(k == ch - 1))
                if ci == 0:
                    nc.scalar.activation(aggblk[b][:], agp[:], COPY)
                else:
                    # drain PSUM fast via Scalar, then add on DVE in SBUF
                    atmp = wp.tile([128, D], f16, tag="atmp")
                    nc.scalar.activation(atmp[:], agp[:], COPY)
                    nc.vector.tensor_tensor(aggblk[b][:], aggblk[b][:],
                                            atmp[:], mybir.AluOpType.add)

            def transpose_to(dst_t, src_sb):
                """dst_t[128, KD, 128] (f16) = src_sb[128, D] transposed."""
                for g in range(KD // TPG):
                    tp = tps.tile([128, TPW], f16, tag="tp")
                    for j in range(TPG):
                        col = (g * TPG + j) * 128
                        nc.tensor.transpose(
                            tp[:, j * 128:(j + 1) * 128],
                            src_sb[:, col:col + 128], ident_t[:])
                    nc.vector.tensor_copy(
                        dst_t[:, g * TPG:(g + 1) * TPG, :].rearrange(
                            "p a b -> p (a b)"), tp[:])

            def dense_block(aggT_t, out_sb, bias_off, relu):
                """out_sb[128, D] = act(aggT.T @ W + b)."""
                for n in range(NT):
                    dp = dps.tile([128, ND], f32, tag="dp")
                    for k in range(KD):
                        nc.tensor.matmul(
                            dp[:], aggT_t[:, k, :], w_t[:, k, n * ND:(n + 1) * ND],
                            start=(k == 0), stop=(k == KD - 1 and not use_bias))
                    if use_bias:
                        nc.tensor.matmul(
                            dp[:], ones_t,
                            brow_t[:, bias_off + n * ND:bias_off + (n + 1) * ND],
                            start=False, stop=True)
                    nc.scalar.activation(out_sb[:, n * ND:(n + 1) * ND], dp[:],
                                         RELU if relu else COPY)

            def dense_and_push(b):
                """Dense + y3 + layer-3 push for L2 block b (from aggblk)."""
                aggT_t = wp.tile([128, KD, 128], f16, tag="aggT")
                transpose_to(aggT_t, aggblk[b])
                x_sb = wp.tile([128, D], f16, tag="x")
                dense_block(aggT_t, x_sb, D, relu=True)
                x3T_t = wp.tile([128, KD, 128], f16, tag="x3T")
                transpose_to(x3T_t, x_sb)
                yp = dps.tile([128, C], f32, tag="dp")
                for k in range(KD):
                    nc.tensor.matmul(yp[:], x3T_t[:, k, :], w3_t[:, k, :],
                                     start=(k == 0), stop=(k == KD - 1))
                y_sb = wp.tile([128, C], f16, tag="y")
                nc.scalar.activation(y_sb[:], yp[:], COPY)
                for h in range(SPH):
                    spt = spp.tile([128, NBINS // SPH, 128], f16, tag="sp")
                    o0 = h * (NBINS // SPH)
                    nc.sync.dma_start(spt[:],
                                      sp3_h[b, :, o0:o0 + NBINS // SPH, :])
                    spv = spt[:].rearrange("p a b -> p (a b)")
                    for m in range(NSP // SPH):
                        cc0 = m * SPW * 128
                        psT = dps.tile([64, SPW * 128], f32, tag="dp")
                        nc.tensor.matmul(psT[:], y_sb[:],
                                         spv[:, cc0:cc0 + SPW * 128],
                                         start=True, stop=True)
                        a0 = o0 * 128 + cc0
                        av = acc_t[:, a0:a0 + SPW * 128]
                        if b == 0:
                            nc.vector.tensor_copy(av, psT[:])
                        else:
                            ptmp = wp.tile([64, SPW * 128], f16, tag="ptmp")
                            nc.scalar.activation(ptmp[:], psT[:], COPY)
                            nc.vector.tensor_tensor(av, av, ptmp[:],
                                                    mybir.AluOpType.add)

            # early layer-2 chunks emitted between AG triggers, sized to
            # each trigger's slack so AG stages fire on schedule
            # after stage-k's trigger at L1 block b, s_blk[0..b+1] are loaded
            EARLY = {
                2: [(b, 0) for b in range(0, 4)],
                3: [(b, 0) for b in range(4, 8)],
                4: [(8, 0), (9, 0), (0, 1), (1, 1), (2, 1)],
            }
            emitted = set()

            # ---------------- layer 1
            for b in range(NBLK):
                agg = spmm_block(b)
                # lazy const loads hidden behind block-0 compute
                if b == 0:
                    nc.sync.dma_start(w_t[:], w12_h[0])
                if b == 1:
                    nc.sync.dma_start(idx23_t[:], idx23_h[:])
                    nc.sync.dma_start(w3_t[:], w3_h[:])
                if b + 1 < NBLK:
                    nc.sync.dma_start(s_blk[b + 1][:],
                                      sker[:, (b + 1) * KT:(b + 2) * KT, :])
                agg_sb = wp.tile([128, D], f16, tag="aggsb")
                nc.scalar.activation(agg_sb[:], agg[:], COPY)
                aggT_t = wp.tile([128, KD, 128], f16, tag="aggT")
                transpose_to(aggT_t, agg_sb)
                x_sb = wp.tile([128, D], f16, tag="x")
                dense_block(aggT_t, x_sb, 0, relu=True)
                nc.sync.dma_start(ag_in[b * 128:(b + 1) * 128, :], x_sb[:])
                if b + 1 in SPL[1:]:
                    st = SPL.index(b + 1)
                    r0 = SPL[st - 1] * 128
                    r1 = (b + 1) * 128
                    nc.gpsimd.collective_compute(
                        "AllGather", mybir.AluOpType.bypass,
                        ins=[ag_in[r0:r1, :]],
                        outs=[ag_out[N_CORES * r0:N_CORES * r1, :]],
                        replica_groups=[list(range(N_CORES))])
                    for bb, ci in EARLY.get(st - 1, []):
                        l2_chunk(bb, ci)
                        emitted.add((bb, ci))
            nc.sync.dma_start(w_t[:], w12_h[1])

            # ---------------- layer 2, slab-arrival phases; L3 push woven in
            LAST = len(CHUNKS) - 1
            for ci in range(LAST):
                for b in range(NBLK):
                    if (b, ci) not in emitted:
                        l2_chunk(b, ci)
            for b in range(NBLK):
                l2_chunk(b, LAST)
                dense_and_push(b)

            # ---------------- layer 3 tail: transpose partialT, ReduceScatter
            GRP = TPW // 64
            for g0 in range(0, NBINS, GRP):
                ps8 = wp.tile([64, GRP * 128], f16, tag="ps128")
                if use_bias:
                    nc.scalar.activation(
                        ps8[:], acc_t[:, g0 * 128:(g0 + GRP) * 128], COPY,
                        bias=b3c_t[:])
                else:
                    nc.scalar.activation(
                        ps8[:], acc_t[:, g0 * 128:(g0 + GRP) * 128], COPY)
                tp = tps.tile([128, TPW], f16, tag="tp")
                for gi in range(GRP):
                    nc.tensor.transpose(tp[:, gi * 64:(gi + 1) * 64],
                                        ps8[:, gi * 128:(gi + 1) * 128],
                                        ident_t[:64, :64])
                ob_sb = wp.tile([128, GRP, C], f16, tag="pt")
                nc.vector.tensor_copy(
                    ob_sb[:].rearrange("p a b -> p (a b)"), tp[:])
                nc.sync.dma_start(
                    part_h[g0 * 128:(g0 + GRP) * 128, :].rearrange(
                        "(a p) c -> p a c", p=128), ob_sb[:])
            nc.gpsimd.collective_compute(
                "ReduceScatter", mybir.AluOpType.add,
                ins=[part_h[:]], outs=[rs_out[:]],
                replica_groups=[list(range(N_CORES))])
            OG = NBLK // 2
            for g0 in range(0, NBLK, OG):
                t16 = wp.tile([128, OG, C], f16, tag="of16")
                nc.sync.dma_start(
                    t16[:], rs_out[g0 * 128:(g0 + OG) * 128, :].rearrange(
                        "(a p) c -> p a c", p=128))
                o_sb = wp.tile([128, OG, C], f32, tag="o")
                nc.vector.tensor_copy(
                    o_sb[:].rearrange("p a b -> p (a b)"),
                    t16[:].rearrange("p a b -> p (a b)"))
                nc.sync.dma_start(
                    out_h[g0 * 128:(g0 + OG) * 128, :].rearrange(
                        "(a p) c -> p a c", p=128), o_sb[:])

    nc.compile()
    return nc


_CACHE = {}


def _get_prog(cfg, kt_blk, use_bias, ranges):
    key = (cfg["N"], cfg["D"], kt_blk, use_bias, ranges)
    if key not in _CACHE:
        _CACHE[key] = _build(cfg, kt_blk, use_bias, ranges)
    return _CACHE[key]


# ---------------------------------------------------------------- entry point
CFG_FULL = dict(N=10000, E=160000, D=1024, C=64, NBLK=10, KT_MIN=4)


def make_in_maps(ins, pp, cfg=None):
    """Per-core input maps (all device tensors fp16)."""
    cfg = cfg or CFG_FULL
    D, C = cfg["D"], cfg["C"]
    KD = D // 128
    KT_TOT = pp["kt_tot"]
    w12 = np.stack([
        np.asarray(ins["W1"], np.float32).reshape(KD, 128, D).transpose(1, 0, 2),
        np.asarray(ins["W2"], np.float32).reshape(KD, 128, D).transpose(1, 0, 2),
    ]).astype(np.float16)
    w3 = (np.asarray(ins["W3"], np.float32).reshape(KD, 128, C)
          .transpose(1, 0, 2).astype(np.float16))
    biases = np.concatenate([
        np.asarray(ins["b1"], np.float32), np.asarray(ins["b2"], np.float32),
        np.asarray(ins["b3"], np.float32), np.ones(128, np.float32),
    ]).astype(np.float16)[None, :]
    ident = np.eye(128, dtype=np.float16)
    b3c = (np.asarray(ins["b3"], np.float32) / N_CORES).reshape(C, 1)
    h16 = np.asarray(ins["h"], np.float32).astype(np.float16)
    maps = []
    for c in range(N_CORES):
        hxp = (h16[pp["nodes_slot"][c]].reshape(KT_TOT, 128, D)
               .transpose(1, 0, 2))
        maps.append(
            dict(hxp=np.ascontiguousarray(hxp),
                 sker=np.ascontiguousarray(pp["S"][c]),
                 idx23=pp["idx23"][c], sp3=pp["sp3"][c],
                 w12=w12, w3=w3, ident=ident, biases=biases, b3c=b3c))
    return maps


def kernel(h, src, dst, W1, b1, W2, b2, W3, b3, cfg=CFG_FULL):
    from concourse.bass_utils import run_bass_kernel_spmd

    h = np.asarray(h, np.float32)
    src = np.asarray(src, np.int32)
    dst = np.asarray(dst, np.int32)
    N, C = cfg["N"], cfg["C"]

    pp = _prep(h, src, dst, cfg)
    use_bias = bool(np.any(b1) or np.any(b2) or np.any(b3))
    nc = _get_prog(cfg, pp["kt_blk"], use_bias, pp["ranges"])

    ins = dict(h=h, W1=W1, b1=b1, W2=W2, b2=b2, W3=W3, b3=b3)
    in_maps = make_in_maps(ins, pp, cfg)
    res = run_bass_kernel_spmd(nc, in_maps, core_ids=list(range(N_CORES)))

    out = np.zeros((N, C), np.float32)
    rows = pp["row_of_node"]
    allout = np.concatenate([res.results[c]["out"] for c in range(N_CORES)],
                            axis=0)
    out[:, :] = allout[rows]
    return out


# revision 26
# speedup vs baseline: 1.0702x; 1.0702x over previous
"""3-layer GCN (DGL GraphConv, norm='both') on 8 Trainium2 NeuronCores.

Strategy (v3):
  - Nodes are packed into 80 balanced bins (128 slots each) by in-degree
    (greedy least-loaded), 10 bins per core -> 1280 padded rows/core.
  - Edges live with the owner (bin) of their dst node. segment_sum runs as
    "scatter matmuls" on the TensorEngine: for each dst block,
    agg[128d, D] += S_kt[128s, 128d].T @ msg_kt[128s, D]. msg rows are the
    DISTINCT src nodes of the block's edges (deduped, ordered by gathered-id
    so each chunk touches a narrow AllGather slab range); S carries
    sum-of-edge-weights norm_src[src]*norm_dst[dst] at (src_slot, dst_slot).
  - Layer 1 does NOT gather: the host pre-packs each core's msg rows
    (h[dedup src]) partition-major, streamed with large contiguous HWDGE
    descriptors. Only layer 2 uses SWDGE dma_gather (whose Q7 descriptor
    generation, ~8.5ns/row, is the scarce resource).
  - Layer-2 gather chunks restrict their source AP to the slab range their
    (gid-sorted) indices touch, with slab-relative indices, so early chunks
    fire while later AllGather stages are still in flight.
  - The on-device datapath is fp16 (PSUM accumulation fp32).
  - Dense W matmuls per dst block: PE-transpose agg -> aggT, then
    x = aggT.T @ W (+ bias via K=1 matmul), ReLU fused into PSUM->SBUF.
  - Layer 3 is PUSH-style, woven into layer 2's block loop (no gather, no
    3rd collective buffer): after y3_j = x2_j @ W3, stream this core's
    block-dense push matrix S'_j from HBM and accumulate
    partialT[64f, all 10240 dst] += y3_j.T-stationary @ S'_j into an SBUF
    fp16 accumulator (DVE adds). After the last block: PE-transpose the 80
    dst column-blocks, one ReduceScatter sums partials across cores, and
    each core writes its own 1280 output rows.
"""
import sys
sys.path.insert(0, '/opt/trn_rl_repo')
import numpy as np

N_CORES = 8
CHMAX = 5        # k-tiles per layer-2 gather chunk


def _ag_splits(nblk):
    """Block-index boundaries of the staged AllGather slabs."""
    if nblk <= 2:
        return [0, nblk]
    fr = [0, round(0.2 * nblk), round(0.4 * nblk), round(0.6 * nblk),
          round(0.8 * nblk), nblk - 1, nblk]
    return sorted(set(b for b in fr if 0 <= b <= nblk))


def _chunks(kt, chmax):
    """Split kt k-tiles into near-equal chunks of size <= chmax."""
    n = -(-kt // chmax)
    base, rem = divmod(kt, n)
    sizes = [base + 1] * rem + [base] * (n - rem)
    out, o = [], 0
    for s in sizes:
        out.append((o, s))
        o += s
    return out


# ---------------------------------------------------------------- host prep
def _partition_nodes(deg_in, n_nodes, nbins):
    """Greedy balanced-edge binning: nodes (sorted by in-degree desc) go to
    the least-loaded bin with a free slot (capacity 128)."""
    import heapq
    order = np.argsort(-deg_in, kind="stable")
    heap = [(0, b) for b in range(nbins)]
    heapq.heapify(heap)
    bin_of = np.empty(n_nodes, np.int32)
    slot_of = np.empty(n_nodes, np.int32)
    count = np.zeros(nbins, np.int64)
    load = np.zeros(nbins, np.int64)
    for n in order:
        while True:
            l, b = heapq.heappop(heap)
            if count[b] < 128:
                break
            # full bin: drop from heap permanently
        bin_of[n] = b
        slot_of[n] = count[b]
        count[b] += 1
        load[b] += int(deg_in[n])
        heapq.heappush(heap, (l + int(deg_in[n]), b))
    return bin_of, slot_of, load


def _prep(h, src, dst, cfg):
    """Build per-core S tiles, gather indices, slot->node maps, per-chunk
    slab ranges, and the layer-3 push matrices."""
    N, E, NBLK = cfg["N"], cfg["E"], cfg["NBLK"]
    nbins = N_CORES * NBLK
    deg_out = np.bincount(src, minlength=N)
    deg_in = np.bincount(dst, minlength=N)
    norm_src = np.clip(deg_out, 1, None).astype(np.float32) ** np.float32(-0.5)
    norm_dst = np.clip(deg_in, 1, None).astype(np.float32) ** np.float32(-0.5)
    w = (norm_src[src] * norm_dst[dst]).astype(np.float32)

    bin_of, slot_of, load = _partition_nodes(deg_in, N, nbins)

    # deal bins to cores snake-wise by load to balance core totals
    order = np.argsort(-load, kind="stable")
    core_of_bin = np.empty(nbins, np.int32)
    blk_of_bin = np.empty(nbins, np.int32)
    nextblk = [0] * N_CORES
    for i, b in enumerate(order):
        r = i // N_CORES
        c = (i % N_CORES) if r % 2 == 0 else (N_CORES - 1 - (i % N_CORES))
        core_of_bin[b] = c
        blk_of_bin[b] = nextblk[c]
        nextblk[c] += 1

    RPC = NBLK * 128
    row_of_node = (core_of_bin[bin_of] * RPC + blk_of_bin[bin_of] * 128
                   + slot_of).astype(np.int32)
    # gather-id layout after the staged slab AllGathers: slab q holds rows
    # [b_q, e_q) of every core, concatenated core-major at offset 8*b_q
    SPL = _ag_splits(NBLK)
    sp = np.array(SPL) * 128
    _c = row_of_node // RPC
    _r = row_of_node % RPC
    _q = np.searchsorted(sp, _r, side="right") - 1
    gid_of_node = (N_CORES * sp[_q] + _c * (sp[_q + 1] - sp[_q])
                   + _r - sp[_q]).astype(np.int32)

    # group edges by dst bin; dedup src nodes per bin, ordered by gid
    ebin = bin_of[dst]
    eorder = np.argsort(ebin, kind="stable")
    counts = np.bincount(ebin, minlength=nbins)
    bounds = np.concatenate([[0], np.cumsum(counts)])
    uniqs = []
    for b in range(nbins):
        es = eorder[bounds[b]:bounds[b + 1]]
        u, inv = np.unique(src[es], return_inverse=True)
        gorder = np.argsort(gid_of_node[u], kind="stable")
        rank = np.empty(len(u), np.int64)
        rank[gorder] = np.arange(len(u))
        uniqs.append((es, u[gorder], rank[inv]))
    max_u = max((len(u) for _, u, _ in uniqs), default=1)
    kt_blk = max(cfg["KT_MIN"], -(-max_u // 128))
    kt_tot = NBLK * kt_blk
    chunks = _chunks(kt_blk, CHMAX)

    # per-(block, chunk) slab-aligned source ranges (shared across cores)
    gbound = N_CORES * sp                     # gid-space slab boundaries
    glo = np.full((NBLK, len(chunks)), gbound[-1], np.int64)
    ghi = np.zeros((NBLK, len(chunks)), np.int64)
    gids_of = {}
    for b in range(nbins):
        es, u, inv = uniqs[b]
        c, blk = int(core_of_bin[b]), int(blk_of_bin[b])
        g = gid_of_node[u]
        gids_of[b] = g
        for ci, (c0, ch) in enumerate(chunks):
            seg = g[c0 * 128:(c0 + ch) * 128]
            if len(seg):
                glo[blk, ci] = min(glo[blk, ci], seg[0])
                ghi[blk, ci] = max(ghi[blk, ci], seg[-1] + 1)
    ranges = []
    for blk in range(NBLK):
        rr = []
        for ci in range(len(chunks)):
            lo = int(gbound[np.searchsorted(gbound, glo[blk, ci],
                                            side="right") - 1])
            hi = int(gbound[np.searchsorted(gbound, ghi[blk, ci],
                                            side="left")])
            if hi <= lo:
                hi = lo + 1024
            rr.append((lo, hi))
        ranges.append(rr)

    nodes_slot = np.zeros((N_CORES, kt_tot * 128), np.int32)
    idx23 = np.zeros((N_CORES, kt_tot * 128), np.int16)
    for blk in range(NBLK):
        for ci, (c0, ch) in enumerate(chunks):
            p0 = blk * kt_blk * 128 + c0 * 128
            idx23[:, p0:p0 + ch * 128] = 0  # relative pad = range base
    S = np.zeros((N_CORES, 128, kt_tot, 128), np.float32)
    for b in range(nbins):
        es, u, inv = uniqs[b]
        c, blk = int(core_of_bin[b]), int(blk_of_bin[b])
        g = gids_of[b]
        p = np.arange(len(u))
        gpos = blk * kt_blk * 128 + p
        nodes_slot[c, gpos] = u
        # slab-relative gather indices per chunk
        rel = np.empty(len(u), np.int64)
        for ci, (c0, ch) in enumerate(chunks):
            s0, s1 = c0 * 128, min((c0 + ch) * 128, len(u))
            if s1 > s0:
                rel[s0:s1] = g[s0:s1] - ranges[blk][ci][0]
        idx23[c, gpos] = rel.astype(np.int16)
        kt = blk * kt_blk + inv // 128
        np.add.at(S[c], (inv % 128, kt, slot_of[dst[es]]), w[es])
    S = S.astype(np.float16)

    # layer-3 push matrices: S'[c][j, src_slot, ob, dst_slot] sums w over
    # edges with src in (core c, block j) and dst in output block ob
    srow = row_of_node[src]
    drow = row_of_node[dst]
    c_u = srow // RPC
    j_u = (srow // 128) % NBLK
    s_u = srow % 128
    ob_v = drow // 128
    d_v = drow % 128
    sp3 = []
    for c in range(N_CORES):
        m = c_u == c
        spc = np.zeros((NBLK, 128, nbins, 128), np.float32)
        np.add.at(spc, (j_u[m], s_u[m], ob_v[m], d_v[m]), w[m])
        sp3.append(spc.astype(np.float16))

    def wrap(ix):  # -> [128, kt_tot*8] wrapped for the 8 Q7 cores
        return np.tile(ix.reshape(-1, 16).T, (8, 1)).copy()

    idx23_w = np.stack([wrap(idx23[c]) for c in range(N_CORES)])
    return dict(S=S, idx23=idx23_w, nodes_slot=nodes_slot, sp3=sp3,
                row_of_node=row_of_node, kt_blk=kt_blk, kt_tot=kt_tot,
                ranges=tuple(tuple(r) for r in ranges))


# ---------------------------------------------------------------- device prog
def _build(cfg, kt_blk, use_bias, ranges):
    import concourse.bacc as bacc
    import concourse.mybir as mybir
    import concourse.tile as tile
    from concourse.library_config import mlp

    f32 = mybir.dt.float32
    f16 = mybir.dt.float16
    i16 = mybir.dt.int16
    RELU = mybir.ActivationFunctionType.Relu
    COPY = mybir.ActivationFunctionType.Copy

    N, D, C, NBLK = cfg["N"], cfg["D"], cfg["C"], cfg["NBLK"]
    NBINS = N_CORES * NBLK
    RPC = NBLK * 128
    NPAD = N_CORES * RPC
    KT = kt_blk
    KT_TOT = NBLK * KT
    CHUNKS = _chunks(KT, CHMAX)
    KD = D // 128               # dense contraction k-tiles
    ND = 512 if D % 512 == 0 else D
    NT = D // ND                # dense n-tiles
    TPW = min(512, D)           # transposes packed per tps tile
    TPG = TPW // 128
    SPL = _ag_splits(NBLK)
    SPW = 512 // 128            # layer-3 push: output blocks per matmul
    NSP = NBINS // SPW          # layer-3 matmuls per source block
    SPH = 2                     # S'_j streamed in halves

    nc = bacc.Bacc("TRN2", target_bir_lowering=False, debug=False,
                   num_devices=N_CORES, num_swdge_queues=4,
                   dynamic_dma_scratch_size=32768)

    hxp = nc.dram_tensor("hxp", [128, KT_TOT, D], f16, kind="ExternalInput")
    sker = nc.dram_tensor("sker", [128, KT_TOT, 128], f16, kind="ExternalInput")
    idx23_h = nc.dram_tensor("idx23", [128, KT_TOT * 8], i16, kind="ExternalInput")
    w12_h = nc.dram_tensor("w12", [2, 128, KD, D], f16, kind="ExternalInput")
    w3_h = nc.dram_tensor("w3", [128, KD, C], f16, kind="ExternalInput")
    ident_h = nc.dram_tensor("ident", [128, 128], f16, kind="ExternalInput")
    bias_h = nc.dram_tensor("biases", [1, 2 * D + C + 128], f16, kind="ExternalInput")
    b3c_h = nc.dram_tensor("b3c", [C, 1], f32, kind="ExternalInput")
    sp3_h = nc.dram_tensor("sp3", [NBLK, 128, NBINS, 128], f16,
                           kind="ExternalInput")
    out_h = nc.dram_tensor("out", [RPC, C], f32, kind="ExternalOutput")

    ag_in = nc.dram_tensor("ag_in", [RPC, D], f16, kind="Internal")
    ag_out = nc.dram_tensor("ag_out", [NPAD, D], f16, kind="Internal",
                            addr_space="Shared")
    part_h = nc.dram_tensor("part", [NPAD, C], f16, kind="Internal")
    rs_out = nc.dram_tensor("rs_out", [RPC, C], f16, kind="Internal")

    with tile.TileContext(nc) as tc:
        nc.gpsimd.load_library(mlp)
        with (
            tc.tile_pool(name="const", bufs=1) as cp,
            tc.tile_pool(name="work", bufs=2) as wp,
            tc.tile_pool(name="msg", bufs=3) as mp,
            tc.tile_pool(name="spstream", bufs=2) as spp,
            tc.tile_pool(name="aggps", bufs=2, space="PSUM") as aps,
            tc.tile_pool(name="densps", bufs=2, space="PSUM") as dps,
            tc.tile_pool(name="tpsps", bufs=2, space="PSUM") as tps,
        ):
            ident_t = cp.tile([128, 128], f16, tag="ident")
            nc.sync.dma_start(ident_t[:], ident_h[:])
            s_blk = [cp.tile([128, KT, 128], f16, tag=f"s{b}", name=f"s_{b}")
                     for b in range(NBLK)]
            nc.sync.dma_start(s_blk[0][:], sker[:, 0:KT, :])
            w_t = cp.tile([128, KD, D], f16, tag="w")
            idx23_t = cp.tile([128, KT_TOT * 8], i16, tag="idx23")
            w3_t = cp.tile([128, KD, C], f16, tag="w3")
            acc_t = cp.tile([64, NBINS * 128], f16, tag="acc")
            aggblk = [cp.tile([128, D], f16, tag=f"ab{b}", name=f"ab_{b}")
                      for b in range(NBLK)]
            if use_bias:
                brow_t = cp.tile([1, 2 * D + C + 128], f16, tag="brow")
                nc.sync.dma_start(brow_t[:], bias_h[:])
                ones_t = brow_t[:, 2 * D + C:2 * D + C + 128]
                b3c_t = cp.tile([C, 1], f32, tag="b3c")
                nc.sync.dma_start(b3c_t[:], b3c_h[:])

            qctr = [0]

            def spmm_block(b):
                """L1: agg[128, D] for dst block b via packed streams + MMs."""
                agg = aps.tile([128, D], f32, tag="aggps")
                nspl = max(1, D // 512)
                for ci, (c0, ch) in enumerate(CHUNKS):
                    msg = mp.tile([128, CHMAX, D], f16, tag="m")
                    nc.sync.dma_start(
                        msg[:, :ch, :],
                        hxp[:, b * KT + c0:b * KT + c0 + ch, :])
                    for k in range(ch):
                        first = (c0 == 0 and k == 0)
                        last = (c0 + ch == KT and k == ch - 1)
                        for n in range(nspl):
                            w0 = n * (D // nspl)
                            w1 = (n + 1) * (D // nspl)
                            nc.tensor.matmul(
                                agg[:, w0:w1], s_blk[b][:, c0 + k, :],
                                msg[:, k, w0:w1],
                                start=first, stop=last)
                return agg

            def l2_chunk(b, ci):
                """L2 chunk: gather (slab-restricted) + MMs, accumulate into
                the block's SBUF partial."""
                c0, ch = CHUNKS[ci]
                agp = aps.tile([128, D], f32, tag="aggps")
                msg = mp.tile([128, CHMAX, D], f16, tag="m")
                lo, hi = ranges[b][ci]
                col0 = (b * KT + c0) * 8
                q = qctr[0] % 4
                qctr[0] += 1
                nc.gpsimd.dma_gather(
                    msg[:, :ch, :], ag_out[lo:hi, :],
                    idx23_t[:, col0:col0 + ch * 8],
                    ch * 128, ch * 128, D, queue_num=q)
                nspl = max(1, D // 512)
                for k in range(ch):
                    for n in range(nspl):
                        w0 = n * (D // nspl)
                        w1 = (n + 1) * (D // nspl)
                        nc.tensor.matmul(
                            agp[:, w0:w1], s_blk[b][:, c0 + k, :],
                            msg[:, k, w0:w1],
                            start=(k == 0), stop=(k == ch - 1))
                if ci == 0:
                    nc.scalar.activation(aggblk[b][:], agp[:], COPY)
                else:
                    nc.vector.tensor_tensor(aggblk[b][:], aggblk[b][:],
                                            agp[:], mybir.AluOpType.add)

            def transpose_to(dst_t, src_sb):
                """dst_t[128, KD, 128] (f16) = src_sb[128, D] transposed."""
                for g in range(KD // TPG):
                    tp = tps.tile([128, TPW], f16, tag="tp")
                    for j in range(TPG):
                        col = (g * TPG + j) * 128
                        nc.tensor.transpose(
                            tp[:, j * 128:(j + 1) * 128],
                            src_sb[:, col:col + 128], ident_t[:])
                    nc.vector.tensor_copy(
                        dst_t[:, g * TPG:(g + 1) * TPG, :].rearrange(
                            "p a b -> p (a b)"), tp[:])

            def dense_block(aggT_t, out_sb, bias_off, relu):
                """out_sb[128, D] = act(aggT.T @ W + b)."""
                for n in range(NT):
                    dp = dps.tile([128, ND], f32, tag="dp")
                    for k in range(KD):
                        nc.tensor.matmul(
                            dp[:], aggT_t[:, k, :], w_t[:, k, n * ND:(n + 1) * ND],
                            start=(k == 0), stop=(k == KD - 1 and not use_bias))
                    if use_bias:
                        nc.tensor.matmul(
                            dp[:], ones_t,
                            brow_t[:, bias_off + n * ND:bias_off + (n + 1) * ND],
                            start=False, stop=True)
                    nc.scalar.activation(out_sb[:, n * ND:(n + 1) * ND], dp[:],
                                         RELU if relu else COPY)

            def dense_and_push(b):
                """Dense + y3 + layer-3 push for L2 block b (from aggblk)."""
                aggT_t = wp.tile([128, KD, 128], f16, tag="aggT")
                transpose_to(aggT_t, aggblk[b])
                x_sb = wp.tile([128, D], f16, tag="x")
                dense_block(aggT_t, x_sb, D, relu=True)
                x3T_t = wp.tile([128, KD, 128], f16, tag="x3T")
                transpose_to(x3T_t, x_sb)
                yp = dps.tile([128, C], f32, tag="dp")
                for k in range(KD):
                    nc.tensor.matmul(yp[:], x3T_t[:, k, :], w3_t[:, k, :],
                                     start=(k == 0), stop=(k == KD - 1))
                y_sb = wp.tile([128, C], f16, tag="y")
                nc.scalar.activation(y_sb[:], yp[:], COPY)
                for h in range(SPH):
                    spt = spp.tile([128, NBINS // SPH, 128], f16, tag="sp")
                    o0 = h * (NBINS // SPH)
                    nc.sync.dma_start(spt[:],
                                      sp3_h[b, :, o0:o0 + NBINS // SPH, :])
                    spv = spt[:].rearrange("p a b -> p (a b)")
                    for m in range(NSP // SPH):
                        cc0 = m * SPW * 128
                        psT = dps.tile([64, SPW * 128], f32, tag="dp")
                        nc.tensor.matmul(psT[:], y_sb[:],
                                         spv[:, cc0:cc0 + SPW * 128],
                                         start=True, stop=True)
                        a0 = o0 * 128 + cc0
                        av = acc_t[:, a0:a0 + SPW * 128]
                        if b == 0:
                            nc.vector.tensor_copy(av, psT[:])
                        else:
                            nc.vector.tensor_tensor(av, av, psT[:],
                                                    mybir.AluOpType.add)

            # ---------------- layer 1
            for b in range(NBLK):
                agg = spmm_block(b)
                # lazy const loads hidden behind block-0 compute
                if b == 0:
                    nc.sync.dma_start(w_t[:], w12_h[0])
                if b == 1:
                    nc.sync.dma_start(idx23_t[:], idx23_h[:])
                    nc.sync.dma_start(w3_t[:], w3_h[:])
                if b + 1 < NBLK:
                    nc.sync.dma_start(s_blk[b + 1][:],
                                      sker[:, (b + 1) * KT:(b + 2) * KT, :])
                agg_sb = wp.tile([128, D], f16, tag="aggsb")
                nc.scalar.activation(agg_sb[:], agg[:], COPY)
                aggT_t = wp.tile([128, KD, 128], f16, tag="aggT")
                transpose_to(aggT_t, agg_sb)
                x_sb = wp.tile([128, D], f16, tag="x")
                dense_block(aggT_t, x_sb, 0, relu=True)
                nc.sync.dma_start(ag_in[b * 128:(b + 1) * 128, :], x_sb[:])
                if b + 1 in SPL[1:]:
                    r0 = SPL[SPL.index(b + 1) - 1] * 128
                    r1 = (b + 1) * 128
                    nc.gpsimd.collective_compute(
                        "AllGather", mybir.AluOpType.bypass,
                        ins=[ag_in[r0:r1, :]],
                        outs=[ag_out[N_CORES * r0:N_CORES * r1, :]],
                        replica_groups=[list(range(N_CORES))])
            nc.sync.dma_start(w_t[:], w12_h[1])

            # ---------------- layer 2, slab-arrival phases; L3 push woven in
            LAST = len(CHUNKS) - 1
            for ci in range(LAST):
                for b in range(NBLK):
                    l2_chunk(b, ci)
            for b in range(NBLK):
                l2_chunk(b, LAST)
                dense_and_push(b)

            # ---------------- layer 3 tail: transpose partialT, ReduceScatter
            GRP = TPW // 64
            for g0 in range(0, NBINS, GRP):
                ps8 = wp.tile([64, GRP * 128], f16, tag="ps128")
                if use_bias:
                    nc.scalar.activation(
                        ps8[:], acc_t[:, g0 * 128:(g0 + GRP) * 128], COPY,
                        bias=b3c_t[:])
                else:
                    nc.scalar.activation(
                        ps8[:], acc_t[:, g0 * 128:(g0 + GRP) * 128], COPY)
                tp = tps.tile([128, TPW], f16, tag="tp")
                for gi in range(GRP):
                    nc.tensor.transpose(tp[:, gi * 64:(gi + 1) * 64],
                                        ps8[:, gi * 128:(gi + 1) * 128],
                                        ident_t[:64, :64])
                ob_sb = wp.tile([128, GRP, C], f16, tag="pt")
                nc.vector.tensor_copy(
                    ob_sb[:].rearrange("p a b -> p (a b)"), tp[:])
                nc.sync.dma_start(
                    part_h[g0 * 128:(g0 + GRP) * 128, :].rearrange(
                        "(a p) c -> p a c", p=128), ob_sb[:])
            nc.gpsimd.collective_compute(
                "ReduceScatter", mybir.AluOpType.add,
                ins=[part_h[:]], outs=[rs_out[:]],
                replica_groups=[list(range(N_CORES))])
            OG = NBLK // 2
            for g0 in range(0, NBLK, OG):
                t16 = wp.tile([128, OG, C], f16, tag="of16")
                nc.sync.dma_start(
                    t16[:], rs_out[g0 * 128:(g0 + OG) * 128, :].rearrange(
                        "(a p) c -> p a c", p=128))
                o_sb = wp.tile([128, OG, C], f32, tag="o")
                nc.vector.tensor_copy(
                    o_sb[:].rearrange("p a b -> p (a b)"),
                    t16[:].rearrange("p a b -> p (a b)"))
                nc.sync.dma_start(
                    out_h[g0 * 128:(g0 + OG) * 128, :].rearrange(
                        "(a p) c -> p a c", p=128), o_sb[:])

    nc.compile()
    return nc


_CACHE = {}


def _get_prog(cfg, kt_blk, use_bias, ranges):
    key = (cfg["N"], cfg["D"], kt_blk, use_bias, ranges)
    if key not in _CACHE:
        _CACHE[key] = _build(cfg, kt_blk, use_bias, ranges)
    return _CACHE[key]


# ---------------------------------------------------------------- entry point
CFG_FULL = dict(N=10000, E=160000, D=1024, C=64, NBLK=10, KT_MIN=4)


def make_in_maps(ins, pp, cfg=None):
    """Per-core input maps (all device tensors fp16)."""
    cfg = cfg or CFG_FULL
    D, C = cfg["D"], cfg["C"]
    KD = D // 128
    KT_TOT = pp["kt_tot"]
    w12 = np.stack([
        np.asarray(ins["W1"], np.float32).reshape(KD, 128, D).transpose(1, 0, 2),
        np.asarray(ins["W2"], np.float32).reshape(KD, 128, D).transpose(1, 0, 2),
    ]).astype(np.float16)
    w3 = (np.asarray(ins["W3"], np.float32).reshape(KD, 128, C)
          .transpose(1, 0, 2).astype(np.float16))
    biases = np.concatenate([
        np.asarray(ins["b1"], np.float32), np.asarray(ins["b2"], np.float32),
        np.asarray(ins["b3"], np.float32), np.ones(128, np.float32),
    ]).astype(np.float16)[None, :]
    ident = np.eye(128, dtype=np.float16)
    b3c = (np.asarray(ins["b3"], np.float32) / N_CORES).reshape(C, 1)
    h16 = np.asarray(ins["h"], np.float32).astype(np.float16)
    maps = []
    for c in range(N_CORES):
        hxp = (h16[pp["nodes_slot"][c]].reshape(KT_TOT, 128, D)
               .transpose(1, 0, 2))
        maps.append(
            dict(hxp=np.ascontiguousarray(hxp),
                 sker=np.ascontiguousarray(pp["S"][c]),
                 idx23=pp["idx23"][c], sp3=pp["sp3"][c],
                 w12=w12, w3=w3, ident=ident, biases=biases, b3c=b3c))
    return maps


def kernel(h, src, dst, W1, b1, W2, b2, W3, b3, cfg=CFG_FULL):
    from concourse.bass_utils import run_bass_kernel_spmd

    h = np.asarray(h, np.float32)
    src = np.asarray(src, np.int32)
    dst = np.asarray(dst, np.int32)
    N, C = cfg["N"], cfg["C"]

    pp = _prep(h, src, dst, cfg)
    use_bias = bool(np.any(b1) or np.any(b2) or np.any(b3))
    nc = _get_prog(cfg, pp["kt_blk"], use_bias, pp["ranges"])

    ins = dict(h=h, W1=W1, b1=b1, W2=W2, b2=b2, W3=W3, b3=b3)
    in_maps = make_in_maps(ins, pp, cfg)
    res = run_bass_kernel_spmd(nc, in_maps, core_ids=list(range(N_CORES)))

    out = np.zeros((N, C), np.float32)
    rows = pp["row_of_node"]
    allout = np.concatenate([res.results[c]["out"] for c in range(N_CORES)],
                            axis=0)
    out[:, :] = allout[rows]
    return out


# revision 29
# speedup vs baseline: 1.0853x; 1.0142x over previous
"""3-layer GCN (DGL GraphConv, norm='both') on 8 Trainium2 NeuronCores.

Strategy (v3):
  - Nodes are packed into 80 balanced bins (128 slots each) by in-degree
    (greedy least-loaded), 10 bins per core -> 1280 padded rows/core.
  - Edges live with the owner (bin) of their dst node. segment_sum runs as
    "scatter matmuls" on the TensorEngine: for each dst block,
    agg[128d, D] += S_kt[128s, 128d].T @ msg_kt[128s, D]. msg rows are the
    DISTINCT src nodes of the block's edges (deduped, ordered by gathered-id
    so each chunk touches a narrow AllGather slab range); S carries
    sum-of-edge-weights norm_src[src]*norm_dst[dst] at (src_slot, dst_slot).
  - Layer 1 does NOT gather: the host pre-packs each core's msg rows
    (h[dedup src]) partition-major, streamed with large contiguous HWDGE
    descriptors. Only layer 2 uses SWDGE dma_gather (whose Q7 descriptor
    generation, ~8.5ns/row, is the scarce resource).
  - Layer-2 gather chunks restrict their source AP to the slab range their
    (gid-sorted) indices touch, with slab-relative indices, so early chunks
    fire while later AllGather stages are still in flight.
  - The on-device datapath is fp16 (PSUM accumulation fp32).
  - Dense W matmuls per dst block: PE-transpose agg -> aggT, then
    x = aggT.T @ W (+ bias via K=1 matmul), ReLU fused into PSUM->SBUF.
  - Layer 3 is PUSH-style, woven into layer 2's block loop (no gather, no
    3rd collective buffer): after y3_j = x2_j @ W3, stream this core's
    block-dense push matrix S'_j from HBM and accumulate
    partialT[64f, all 10240 dst] += y3_j.T-stationary @ S'_j into an SBUF
    fp16 accumulator (DVE adds). After the last block: PE-transpose the 80
    dst column-blocks, one ReduceScatter sums partials across cores, and
    each core writes its own 1280 output rows.
"""
import sys
sys.path.insert(0, '/opt/trn_rl_repo')
import numpy as np

N_CORES = 8
CHMAX = 5        # k-tiles per layer-2 gather chunk


def _ag_splits(nblk):
    """Block-index boundaries of the staged AllGather slabs."""
    if nblk <= 2:
        return [0, nblk]
    fr = [0, round(0.2 * nblk), round(0.4 * nblk), round(0.6 * nblk),
          round(0.8 * nblk), nblk]
    return sorted(set(b for b in fr if 0 <= b <= nblk))


def _chunks(kt, chmax):
    """Split kt k-tiles into near-equal chunks of size <= chmax."""
    n = -(-kt // chmax)
    base, rem = divmod(kt, n)
    sizes = [base + 1] * rem + [base] * (n - rem)
    out, o = [], 0
    for s in sizes:
        out.append((o, s))
        o += s
    return out


# ---------------------------------------------------------------- host prep
def _partition_nodes(deg_in, n_nodes, nbins):
    """Greedy balanced-edge binning: nodes (sorted by in-degree desc) go to
    the least-loaded bin with a free slot (capacity 128)."""
    import heapq
    order = np.argsort(-deg_in, kind="stable")
    heap = [(0, b) for b in range(nbins)]
    heapq.heapify(heap)
    bin_of = np.empty(n_nodes, np.int32)
    slot_of = np.empty(n_nodes, np.int32)
    count = np.zeros(nbins, np.int64)
    load = np.zeros(nbins, np.int64)
    for n in order:
        while True:
            l, b = heapq.heappop(heap)
            if count[b] < 128:
                break
            # full bin: drop from heap permanently
        bin_of[n] = b
        slot_of[n] = count[b]
        count[b] += 1
        load[b] += int(deg_in[n])
        heapq.heappush(heap, (l + int(deg_in[n]), b))
    return bin_of, slot_of, load


def _prep(h, src, dst, cfg):
    """Build per-core S tiles, gather indices, slot->node maps, per-chunk
    slab ranges, and the layer-3 push matrices."""
    N, E, NBLK = cfg["N"], cfg["E"], cfg["NBLK"]
    nbins = N_CORES * NBLK
    deg_out = np.bincount(src, minlength=N)
    deg_in = np.bincount(dst, minlength=N)
    norm_src = np.clip(deg_out, 1, None).astype(np.float32) ** np.float32(-0.5)
    norm_dst = np.clip(deg_in, 1, None).astype(np.float32) ** np.float32(-0.5)
    w = (norm_src[src] * norm_dst[dst]).astype(np.float32)

    bin_of, slot_of, load = _partition_nodes(deg_in, N, nbins)

    # deal bins to cores snake-wise by load to balance core totals
    order = np.argsort(-load, kind="stable")
    core_of_bin = np.empty(nbins, np.int32)
    blk_of_bin = np.empty(nbins, np.int32)
    nextblk = [0] * N_CORES
    for i, b in enumerate(order):
        r = i // N_CORES
        c = (i % N_CORES) if r % 2 == 0 else (N_CORES - 1 - (i % N_CORES))
        core_of_bin[b] = c
        blk_of_bin[b] = nextblk[c]
        nextblk[c] += 1

    RPC = NBLK * 128
    row_of_node = (core_of_bin[bin_of] * RPC + blk_of_bin[bin_of] * 128
                   + slot_of).astype(np.int32)
    # gather-id layout after the staged slab AllGathers: slab q holds rows
    # [b_q, e_q) of every core, concatenated core-major at offset 8*b_q
    SPL = _ag_splits(NBLK)
    sp = np.array(SPL) * 128
    _c = row_of_node // RPC
    _r = row_of_node % RPC
    _q = np.searchsorted(sp, _r, side="right") - 1
    gid_of_node = (N_CORES * sp[_q] + _c * (sp[_q + 1] - sp[_q])
                   + _r - sp[_q]).astype(np.int32)

    # group edges by dst bin; dedup src nodes per bin, ordered by gid
    ebin = bin_of[dst]
    eorder = np.argsort(ebin, kind="stable")
    counts = np.bincount(ebin, minlength=nbins)
    bounds = np.concatenate([[0], np.cumsum(counts)])
    uniqs = []
    for b in range(nbins):
        es = eorder[bounds[b]:bounds[b + 1]]
        u, inv = np.unique(src[es], return_inverse=True)
        gorder = np.argsort(gid_of_node[u], kind="stable")
        rank = np.empty(len(u), np.int64)
        rank[gorder] = np.arange(len(u))
        uniqs.append((es, u[gorder], rank[inv]))
    max_u = max((len(u) for _, u, _ in uniqs), default=1)
    kt_blk = max(cfg["KT_MIN"], -(-max_u // 128))
    kt_tot = NBLK * kt_blk
    chunks = _chunks(kt_blk, CHMAX)

    # per-(block, chunk) slab-aligned source ranges (shared across cores)
    gbound = N_CORES * sp                     # gid-space slab boundaries
    glo = np.full((NBLK, len(chunks)), gbound[-1], np.int64)
    ghi = np.zeros((NBLK, len(chunks)), np.int64)
    gids_of = {}
    for b in range(nbins):
        es, u, inv = uniqs[b]
        c, blk = int(core_of_bin[b]), int(blk_of_bin[b])
        g = gid_of_node[u]
        gids_of[b] = g
        for ci, (c0, ch) in enumerate(chunks):
            seg = g[c0 * 128:(c0 + ch) * 128]
            if len(seg):
                glo[blk, ci] = min(glo[blk, ci], seg[0])
                ghi[blk, ci] = max(ghi[blk, ci], seg[-1] + 1)
    ranges = []
    for blk in range(NBLK):
        rr = []
        for ci in range(len(chunks)):
            lo = int(gbound[np.searchsorted(gbound, glo[blk, ci],
                                            side="right") - 1])
            hi = int(gbound[np.searchsorted(gbound, ghi[blk, ci],
                                            side="left")])
            if hi <= lo:
                hi = lo + 1024
            rr.append((lo, hi))
        ranges.append(rr)

    nodes_slot = np.zeros((N_CORES, kt_tot * 128), np.int32)
    idx23 = np.zeros((N_CORES, kt_tot * 128), np.int16)
    for blk in range(NBLK):
        for ci, (c0, ch) in enumerate(chunks):
            p0 = blk * kt_blk * 128 + c0 * 128
            idx23[:, p0:p0 + ch * 128] = 0  # relative pad = range base
    S = np.zeros((N_CORES, 128, kt_tot, 128), np.float32)
    for b in range(nbins):
        es, u, inv = uniqs[b]
        c, blk = int(core_of_bin[b]), int(blk_of_bin[b])
        g = gids_of[b]
        p = np.arange(len(u))
        gpos = blk * kt_blk * 128 + p
        nodes_slot[c, gpos] = u
        # slab-relative gather indices per chunk
        rel = np.empty(len(u), np.int64)
        for ci, (c0, ch) in enumerate(chunks):
            s0, s1 = c0 * 128, min((c0 + ch) * 128, len(u))
            if s1 > s0:
                rel[s0:s1] = g[s0:s1] - ranges[blk][ci][0]
        idx23[c, gpos] = rel.astype(np.int16)
        kt = blk * kt_blk + inv // 128
        np.add.at(S[c], (inv % 128, kt, slot_of[dst[es]]), w[es])
    S = S.astype(np.float16)

    # layer-3 push matrices: S'[c][j, src_slot, ob, dst_slot] sums w over
    # edges with src in (core c, block j) and dst in output block ob
    srow = row_of_node[src]
    drow = row_of_node[dst]
    c_u = srow // RPC
    j_u = (srow // 128) % NBLK
    s_u = srow % 128
    ob_v = drow // 128
    d_v = drow % 128
    sp3 = []
    for c in range(N_CORES):
        m = c_u == c
        spc = np.zeros((NBLK, 128, nbins, 128), np.float32)
        np.add.at(spc, (j_u[m], s_u[m], ob_v[m], d_v[m]), w[m])
        sp3.append(spc.astype(np.float16))

    def wrap(ix):  # -> [128, kt_tot*8] wrapped for the 8 Q7 cores
        return np.tile(ix.reshape(-1, 16).T, (8, 1)).copy()

    idx23_w = np.stack([wrap(idx23[c]) for c in range(N_CORES)])
    return dict(S=S, idx23=idx23_w, nodes_slot=nodes_slot, sp3=sp3,
                row_of_node=row_of_node, kt_blk=kt_blk, kt_tot=kt_tot,
                ranges=tuple(tuple(r) for r in ranges))


# ---------------------------------------------------------------- device prog
def _build(cfg, kt_blk, use_bias, ranges):
    import concourse.bacc as bacc
    import concourse.mybir as mybir
    import concourse.tile as tile
    from concourse.library_config import mlp

    f32 = mybir.dt.float32
    f16 = mybir.dt.float16
    i16 = mybir.dt.int16
    RELU = mybir.ActivationFunctionType.Relu
    COPY = mybir.ActivationFunctionType.Copy

    N, D, C, NBLK = cfg["N"], cfg["D"], cfg["C"], cfg["NBLK"]
    NBINS = N_CORES * NBLK
    RPC = NBLK * 128
    NPAD = N_CORES * RPC
    KT = kt_blk
    KT_TOT = NBLK * KT
    CHUNKS = _chunks(KT, CHMAX)
    KD = D // 128               # dense contraction k-tiles
    ND = 512 if D % 512 == 0 else D
    NT = D // ND                # dense n-tiles
    TPW = min(512, D)           # transposes packed per tps tile
    TPG = TPW // 128
    SPL = _ag_splits(NBLK)
    SPW = 512 // 128            # layer-3 push: output blocks per matmul
    NSP = NBINS // SPW          # layer-3 matmuls per source block
    SPH = 2                     # S'_j streamed in halves

    nc = bacc.Bacc("TRN2", target_bir_lowering=False, debug=False,
                   num_devices=N_CORES, num_swdge_queues=4,
                   dynamic_dma_scratch_size=32768)

    hxp = nc.dram_tensor("hxp", [128, KT_TOT, D], f16, kind="ExternalInput")
    sker = nc.dram_tensor("sker", [128, KT_TOT, 128], f16, kind="ExternalInput")
    idx23_h = nc.dram_tensor("idx23", [128, KT_TOT * 8], i16, kind="ExternalInput")
    w12_h = nc.dram_tensor("w12", [2, 128, KD, D], f16, kind="ExternalInput")
    w3_h = nc.dram_tensor("w3", [128, KD, C], f16, kind="ExternalInput")
    ident_h = nc.dram_tensor("ident", [128, 128], f16, kind="ExternalInput")
    bias_h = nc.dram_tensor("biases", [1, 2 * D + C + 128], f16, kind="ExternalInput")
    b3c_h = nc.dram_tensor("b3c", [C, 1], f32, kind="ExternalInput")
    sp3_h = nc.dram_tensor("sp3", [NBLK, 128, NBINS, 128], f16,
                           kind="ExternalInput")
    out_h = nc.dram_tensor("out", [RPC, C], f32, kind="ExternalOutput")

    ag_in = nc.dram_tensor("ag_in", [RPC, D], f16, kind="Internal")
    ag_out = nc.dram_tensor("ag_out", [NPAD, D], f16, kind="Internal",
                            addr_space="Shared")
    part_h = nc.dram_tensor("part", [NPAD, C], f16, kind="Internal")
    rs_out = nc.dram_tensor("rs_out", [RPC, C], f16, kind="Internal")

    with tile.TileContext(nc) as tc:
        nc.gpsimd.load_library(mlp)
        with (
            tc.tile_pool(name="const", bufs=1) as cp,
            tc.tile_pool(name="work", bufs=2) as wp,
            tc.tile_pool(name="msg", bufs=4) as mp,
            tc.tile_pool(name="spstream", bufs=2) as spp,
            tc.tile_pool(name="aggps", bufs=2, space="PSUM") as aps,
            tc.tile_pool(name="densps", bufs=2, space="PSUM") as dps,
            tc.tile_pool(name="tpsps", bufs=2, space="PSUM") as tps,
        ):
            ident_t = cp.tile([128, 128], f16, tag="ident")
            nc.sync.dma_start(ident_t[:], ident_h[:])
            s_blk = [cp.tile([128, KT, 128], f16, tag=f"s{b}", name=f"s_{b}")
                     for b in range(NBLK)]
            nc.sync.dma_start(s_blk[0][:], sker[:, 0:KT, :])
            w_t = cp.tile([128, KD, D], f16, tag="w")
            idx23_t = cp.tile([128, KT_TOT * 8], i16, tag="idx23")
            w3_t = cp.tile([128, KD, C], f16, tag="w3")
            acc_t = cp.tile([64, NBINS * 128], f16, tag="acc")
            aggblk = [cp.tile([128, D], f16, tag=f"ab{b}", name=f"ab_{b}")
                      for b in range(NBLK)]
            if use_bias:
                brow_t = cp.tile([1, 2 * D + C + 128], f16, tag="brow")
                nc.sync.dma_start(brow_t[:], bias_h[:])
                ones_t = brow_t[:, 2 * D + C:2 * D + C + 128]
                b3c_t = cp.tile([C, 1], f32, tag="b3c")
                nc.sync.dma_start(b3c_t[:], b3c_h[:])

            qctr = [0]

            def spmm_block(b):
                """L1: agg[128, D] for dst block b via packed streams + MMs."""
                agg = aps.tile([128, D], f32, tag="aggps")
                nspl = max(1, D // 512)
                for ci, (c0, ch) in enumerate(CHUNKS):
                    msg = mp.tile([128, CHMAX, D], f16, tag="m")
                    nc.sync.dma_start(
                        msg[:, :ch, :],
                        hxp[:, b * KT + c0:b * KT + c0 + ch, :])
                    for k in range(ch):
                        first = (c0 == 0 and k == 0)
                        last = (c0 + ch == KT and k == ch - 1)
                        for n in range(nspl):
                            w0 = n * (D // nspl)
                            w1 = (n + 1) * (D // nspl)
                            nc.tensor.matmul(
                                agg[:, w0:w1], s_blk[b][:, c0 + k, :],
                                msg[:, k, w0:w1],
                                start=first, stop=last)
                return agg

            def l2_chunk(b, ci):
                """L2 chunk: gather (slab-restricted) + MMs, accumulate into
                the block's SBUF partial."""
                c0, ch = CHUNKS[ci]
                agp = aps.tile([128, D], f32, tag="aggps")
                msg = mp.tile([128, CHMAX, D], f16, tag="m")
                lo, hi = ranges[b][ci]
                col0 = (b * KT + c0) * 8
                q = qctr[0] % 4
                qctr[0] += 1
                nc.gpsimd.dma_gather(
                    msg[:, :ch, :], ag_out[lo:hi, :],
                    idx23_t[:, col0:col0 + ch * 8],
                    ch * 128, ch * 128, D, queue_num=q)
                nspl = max(1, D // 512)
                for k in range(ch):
                    for n in range(nspl):
                        w0 = n * (D // nspl)
                        w1 = (n + 1) * (D // nspl)
                        nc.tensor.matmul(
                            agp[:, w0:w1], s_blk[b][:, c0 + k, :],
                            msg[:, k, w0:w1],
                            start=(k == 0), stop=(k == ch - 1))
                if ci == 0:
                    nc.scalar.activation(aggblk[b][:], agp[:], COPY)
                else:
                    nc.vector.tensor_tensor(aggblk[b][:], aggblk[b][:],
                                            agp[:], mybir.AluOpType.add)

            def transpose_to(dst_t, src_sb):
                """dst_t[128, KD, 128] (f16) = src_sb[128, D] transposed."""
                for g in range(KD // TPG):
                    tp = tps.tile([128, TPW], f16, tag="tp")
                    for j in range(TPG):
                        col = (g * TPG + j) * 128
                        nc.tensor.transpose(
                            tp[:, j * 128:(j + 1) * 128],
                            src_sb[:, col:col + 128], ident_t[:])
                    nc.vector.tensor_copy(
                        dst_t[:, g * TPG:(g + 1) * TPG, :].rearrange(
                            "p a b -> p (a b)"), tp[:])

            def dense_block(aggT_t, out_sb, bias_off, relu):
                """out_sb[128, D] = act(aggT.T @ W + b)."""
                for n in range(NT):
                    dp = dps.tile([128, ND], f32, tag="dp")
                    for k in range(KD):
                        nc.tensor.matmul(
                            dp[:], aggT_t[:, k, :], w_t[:, k, n * ND:(n + 1) * ND],
                            start=(k == 0), stop=(k == KD - 1 and not use_bias))
                    if use_bias:
                        nc.tensor.matmul(
                            dp[:], ones_t,
                            brow_t[:, bias_off + n * ND:bias_off + (n + 1) * ND],
                            start=False, stop=True)
                    nc.scalar.activation(out_sb[:, n * ND:(n + 1) * ND], dp[:],
                                         RELU if relu else COPY)

            def dense_and_push(b):
                """Dense + y3 + layer-3 push for L2 block b (from aggblk)."""
                aggT_t = wp.tile([128, KD, 128], f16, tag="aggT")
                transpose_to(aggT_t, aggblk[b])
                x_sb = wp.tile([128, D], f16, tag="x")
                dense_block(aggT_t, x_sb, D, relu=True)
                x3T_t = wp.tile([128, KD, 128], f16, tag="x3T")
                transpose_to(x3T_t, x_sb)
                yp = dps.tile([128, C], f32, tag="dp")
                for k in range(KD):
                    nc.tensor.matmul(yp[:], x3T_t[:, k, :], w3_t[:, k, :],
                                     start=(k == 0), stop=(k == KD - 1))
                y_sb = wp.tile([128, C], f16, tag="y")
                nc.scalar.activation(y_sb[:], yp[:], COPY)
                for h in range(SPH):
                    spt = spp.tile([128, NBINS // SPH, 128], f16, tag="sp")
                    o0 = h * (NBINS // SPH)
                    nc.sync.dma_start(spt[:],
                                      sp3_h[b, :, o0:o0 + NBINS // SPH, :])
                    spv = spt[:].rearrange("p a b -> p (a b)")
                    for m in range(NSP // SPH):
                        cc0 = m * SPW * 128
                        psT = dps.tile([64, SPW * 128], f32, tag="dp")
                        nc.tensor.matmul(psT[:], y_sb[:],
                                         spv[:, cc0:cc0 + SPW * 128],
                                         start=True, stop=True)
                        a0 = o0 * 128 + cc0
                        av = acc_t[:, a0:a0 + SPW * 128]
                        if b == 0:
                            nc.vector.tensor_copy(av, psT[:])
                        else:
                            nc.vector.tensor_tensor(av, av, psT[:],
                                                    mybir.AluOpType.add)

            # ---------------- layer 1
            for b in range(NBLK):
                agg = spmm_block(b)
                # lazy const loads hidden behind block-0 compute
                if b == 0:
                    nc.sync.dma_start(w_t[:], w12_h[0])
                if b == 1:
                    nc.sync.dma_start(idx23_t[:], idx23_h[:])
                    nc.sync.dma_start(w3_t[:], w3_h[:])
                if b + 1 < NBLK:
                    nc.sync.dma_start(s_blk[b + 1][:],
                                      sker[:, (b + 1) * KT:(b + 2) * KT, :])
                agg_sb = wp.tile([128, D], f16, tag="aggsb")
                nc.scalar.activation(agg_sb[:], agg[:], COPY)
                aggT_t = wp.tile([128, KD, 128], f16, tag="aggT")
                transpose_to(aggT_t, agg_sb)
                x_sb = wp.tile([128, D], f16, tag="x")
                dense_block(aggT_t, x_sb, 0, relu=True)
                nc.sync.dma_start(ag_in[b * 128:(b + 1) * 128, :], x_sb[:])
                if b + 1 in SPL[1:]:
                    r0 = SPL[SPL.index(b + 1) - 1] * 128
                    r1 = (b + 1) * 128
                    nc.gpsimd.collective_compute(
                        "AllGather", mybir.AluOpType.bypass,
                        ins=[ag_in[r0:r1, :]],
                        outs=[ag_out[N_CORES * r0:N_CORES * r1, :]],
                        replica_groups=[list(range(N_CORES))])
            nc.sync.dma_start(w_t[:], w12_h[1])

            # ---------------- layer 2, slab-arrival phases; L3 push woven in
            LAST = len(CHUNKS) - 1
            for ci in range(LAST):
                for b in range(NBLK):
                    l2_chunk(b, ci)
            for b in range(NBLK):
                l2_chunk(b, LAST)
                dense_and_push(b)

            # ---------------- layer 3 tail: transpose partialT, ReduceScatter
            GRP = TPW // 64
            for g0 in range(0, NBINS, GRP):
                ps8 = wp.tile([64, GRP * 128], f16, tag="ps128")
                if use_bias:
                    nc.scalar.activation(
                        ps8[:], acc_t[:, g0 * 128:(g0 + GRP) * 128], COPY,
                        bias=b3c_t[:])
                else:
                    nc.scalar.activation(
                        ps8[:], acc_t[:, g0 * 128:(g0 + GRP) * 128], COPY)
                tp = tps.tile([128, TPW], f16, tag="tp")
                for gi in range(GRP):
                    nc.tensor.transpose(tp[:, gi * 64:(gi + 1) * 64],
                                        ps8[:, gi * 128:(gi + 1) * 128],
                                        ident_t[:64, :64])
                ob_sb = wp.tile([128, GRP, C], f16, tag="pt")
                nc.vector.tensor_copy(
                    ob_sb[:].rearrange("p a b -> p (a b)"), tp[:])
                nc.sync.dma_start(
                    part_h[g0 * 128:(g0 + GRP) * 128, :].rearrange(
                        "(a p) c -> p a c", p=128), ob_sb[:])
            nc.gpsimd.collective_compute(
                "ReduceScatter", mybir.AluOpType.add,
                ins=[part_h[:]], outs=[rs_out[:]],
                replica_groups=[list(range(N_CORES))])
            OG = NBLK // 2
            for g0 in range(0, NBLK, OG):
                t16 = wp.tile([128, OG, C], f16, tag="of16")
                nc.sync.dma_start(
                    t16[:], rs_out[g0 * 128:(g0 + OG) * 128, :].rearrange(
                        "(a p) c -> p a c", p=128))
                o_sb = wp.tile([128, OG, C], f32, tag="o")
                nc.vector.tensor_copy(
                    o_sb[:].rearrange("p a b -> p (a b)"),
                    t16[:].rearrange("p a b -> p (a b)"))
                nc.sync.dma_start(
                    out_h[g0 * 128:(g0 + OG) * 128, :].rearrange(
                        "(a p) c -> p a c", p=128), o_sb[:])

    nc.compile()
    return nc


_CACHE = {}


def _get_prog(cfg, kt_blk, use_bias, ranges):
    key = (cfg["N"], cfg["D"], kt_blk, use_bias, ranges)
    if key not in _CACHE:
        _CACHE[key] = _build(cfg, kt_blk, use_bias, ranges)
    return _CACHE[key]


# ---------------------------------------------------------------- entry point
CFG_FULL = dict(N=10000, E=160000, D=1024, C=64, NBLK=10, KT_MIN=4)


def make_in_maps(ins, pp, cfg=None):
    """Per-core input maps (all device tensors fp16)."""
    cfg = cfg or CFG_FULL
    D, C = cfg["D"], cfg["C"]
    KD = D // 128
    KT_TOT = pp["kt_tot"]
    w12 = np.stack([
        np.asarray(ins["W1"], np.float32).reshape(KD, 128, D).transpose(1, 0, 2),
        np.asarray(ins["W2"], np.float32).reshape(KD, 128, D).transpose(1, 0, 2),
    ]).astype(np.float16)
    w3 = (np.asarray(ins["W3"], np.float32).reshape(KD, 128, C)
          .transpose(1, 0, 2).astype(np.float16))
    biases = np.concatenate([
        np.asarray(ins["b1"], np.float32), np.asarray(ins["b2"], np.float32),
        np.asarray(ins["b3"], np.float32), np.ones(128, np.float32),
    ]).astype(np.float16)[None, :]
    ident = np.eye(128, dtype=np.float16)
    b3c = (np.asarray(ins["b3"], np.float32) / N_CORES).reshape(C, 1)
    h16 = np.asarray(ins["h"], np.float32).astype(np.float16)
    maps = []
    for c in range(N_CORES):
        hxp = (h16[pp["nodes_slot"][c]].reshape(KT_TOT, 128, D)
               .transpose(1, 0, 2))
        maps.append(
            dict(hxp=np.ascontiguousarray(hxp),
                 sker=np.ascontiguousarray(pp["S"][c]),
                 idx23=pp["idx23"][c], sp3=pp["sp3"][c],
                 w12=w12, w3=w3, ident=ident, biases=biases, b3c=b3c))
    return maps


def kernel(h, src, dst, W1, b1, W2, b2, W3, b3, cfg=CFG_FULL):
    from concourse.bass_utils import run_bass_kernel_spmd

    h = np.asarray(h, np.float32)
    src = np.asarray(src, np.int32)
    dst = np.asarray(dst, np.int32)
    N, C = cfg["N"], cfg["C"]

    pp = _prep(h, src, dst, cfg)
    use_bias = bool(np.any(b1) or np.any(b2) or np.any(b3))
    nc = _get_prog(cfg, pp["kt_blk"], use_bias, pp["ranges"])

    ins = dict(h=h, W1=W1, b1=b1, W2=W2, b2=b2, W3=W3, b3=b3)
    in_maps = make_in_maps(ins, pp, cfg)
    res = run_bass_kernel_spmd(nc, in_maps, core_ids=list(range(N_CORES)))

    out = np.zeros((N, C), np.float32)
    rows = pp["row_of_node"]
    allout = np.concatenate([res.results[c]["out"] for c in range(N_CORES)],
                            axis=0)
    out[:, :] = allout[rows]
    return out
